# revision 10
# baseline (speedup 1.0000x reference)
"""FeatureAlign_V2 Bass kernel for 8 NeuronCores (TRN2).

Sharding: core c handles batch b = c//2 and output image rows
[h0, h0+64) with h0 = 64*(c%2). One SPMD program for all cores; all
h0-dependence is carried by per-core input tensors.
"""
import sys, os

for _p in ("/opt/trn_rl_repo", "/root/.axon_site/_ro/trn_rl_repo"):
    if os.path.isdir(_p) and _p not in sys.path:
        sys.path.insert(0, _p)

import numpy as np
from contextlib import ExitStack

_BASS = {}
F32 = F32R = BF16 = I16 = AX = AF = None


def _lazy_imports():
    global F32, F32R, BF16, I16, AX, AF
    if _BASS:
        return
    import concourse.bass as bass
    import concourse.tile as tile
    from concourse import bacc, mybir
    from concourse.bass_utils import run_bass_kernel_spmd
    _BASS.update(bass=bass, tile=tile, bacc=bacc, mybir=mybir,
                 run=run_bass_kernel_spmd)
    F32 = mybir.dt.float32
    F32R = mybir.dt.float32r
    BF16 = mybir.dt.bfloat16
    I16 = mybir.dt.int16
    AX = mybir.AluOpType
    AF = mybir.ActivationFunctionType


N_CORES = 8
B, G, CG = 4, 8, 16
H = W = 128
HS = WS = 64
NC = 128
PAD = 5
BROW = 64 + 2 * PAD      # 74
BCOL = W + 2 * PAD       # 138
NELEM = BROW * BCOL      # 10212
NPIX = 64 * W            # 8192
CHUNK = 512
NCHUNK = NPIX // CHUNK
T = 9
KY = [t // 3 - 1 for t in range(T)]
KX = [t % 3 - 1 for t in range(T)]
SROWS = 40               # feat_s band source rows

_PROG = None


# --------------------------------------------------------------------------
# host-side prep
# --------------------------------------------------------------------------
def _orig_ch(q, g, t):
    if q < 2:
        return g * (2 * T) + t * 2 + q
    return 2 * T * G + g * T + t


def _make_consts(fsm_atten_w, fsm_conv_w, om_w, om_b, dcn_w, dcn_b):
    import ml_dtypes
    bf = ml_dtypes.bfloat16
    c = {}
    c["att_lhsT"] = np.ascontiguousarray(fsm_atten_w.T / float(H * W)).astype(np.float32)
    c["arm_lhsT"] = np.ascontiguousarray(fsm_conv_w.T).astype(np.float32)

    omA = np.zeros((128, 3 * 72), np.float32)
    omB = np.zeros((128, 3 * 72), np.float32)
    bias = np.zeros((72, 3), np.float32)
    for q in range(3):
        for t in range(T):
            for g in range(G):
                ch = _orig_ch(q, g, t)
                r = t * 8 + g
                omA[:, q * 72 + r] = om_w[ch, :128]
                omB[:, q * 72 + r] = 2.0 * om_w[ch, 128:]
                bias[r, q] = om_b[ch] + (4.0 if q < 2 else 0.0)
    c["om_lhsT_A"] = omA.astype(bf)
    c["om_lhsT_B"] = omB.astype(bf)
    c["om_bias"] = bias

    wt = np.zeros((128, T * 128), np.float32)
    for t in range(T):
        for g in range(G):
            for ci in range(CG):
                for o in range(CG):
                    wt[g * CG + ci, t * 128 + g * CG + o] = \
                        dcn_w[g * CG + o, ci, KY[t] + 1, KX[t] + 1]
    c["wt_lhsT"] = wt.astype(bf)

    sel = np.zeros((72, T * 128), np.float32)
    for t in range(T):
        for g in range(G):
            sel[t * 8 + g, t * 128 + g * CG:t * 128 + (g + 1) * CG] = 1.0
    c["sel_lhsT"] = sel.astype(bf)

    c["dcn_bias"] = dcn_b.reshape(128, 1).astype(np.float32)

    hl = np.arange(NPIX, dtype=np.float32) // W
    wp = np.arange(NPIX, dtype=np.float32) % W
    basep = np.zeros((72, NPIX), np.float32)
    for t in range(T):
        v = (hl + KY[t] + PAD - 4) * BCOL + (wp + KX[t] + PAD - 4)
        basep[t * 8:(t + 1) * 8] = v
    c["basep"] = basep
    return c


def _per_core_inputs(inputs, consts, core):
    b, h0 = core // 2, 64 * (core % 2)
    fl = np.asarray(inputs["feat_l"][b], np.float32).reshape(NC, H * W)
    ours = fl[:, h0 * W:(h0 + 64) * W]
    other = fl[:, (64 - h0) * W:(128 - h0) * W]
    feat_l = np.ascontiguousarray(np.concatenate([ours, other], axis=1))

    fs = np.asarray(inputs["feat_s"][b], np.float32).reshape(NC, HS, WS)
    rows = np.clip(h0 // 2 - 4 + np.arange(SROWS), 0, HS - 1)
    feat_s_band = np.ascontiguousarray(fs[:, rows, :]).reshape(NC, SROWS * WS)

    lo = float(max(0, 5 - h0) * BCOL)
    hi = float(min(BROW, 133 - h0) * BCOL - 1)
    lohi = np.zeros((72, 2), np.float32)
    lohi[:, 0] = lo
    lohi[:, 1] = hi

    m = dict(consts)
    m["feat_l"] = feat_l
    m["feat_s_band"] = feat_s_band
    m["lohi"] = lohi
    return m


# --------------------------------------------------------------------------
# device program
# --------------------------------------------------------------------------
def _build():
    _lazy_imports()
    bacc, tile = _BASS["bacc"], _BASS["tile"]
    nc = bacc.Bacc("TRN2", target_bir_lowering=False, debug=False,
                   num_devices=N_CORES)
    d = {}

    def din(name, shape, dt):
        d[name] = nc.dram_tensor(name, shape, dt, kind="ExternalInput").ap()

    din("feat_l", [NC, H * W], F32)
    din("feat_s_band", [NC, SROWS * WS], F32)
    din("att_lhsT", [128, 128], F32)
    din("arm_lhsT", [128, 128], F32)
    din("om_lhsT_A", [128, 216], BF16)
    din("om_lhsT_B", [128, 216], BF16)
    din("om_bias", [72, 3], F32)
    din("wt_lhsT", [128, T * 128], BF16)
    din("sel_lhsT", [72, T * 128], BF16)
    din("dcn_bias", [128, 1], F32)
    din("basep", [72, NPIX], F32)
    din("lohi", [72, 2], F32)
    y = nc.dram_tensor("y", [NC, NPIX], F32, kind="ExternalOutput").ap()

    with tile.TileContext(nc) as tc, ExitStack() as ctx:
        _body(nc, tc, ctx, d, y)
    nc.compile()
    return nc


def _body(nc, tc, ctx, d, y):
    mybir = _BASS["mybir"]
    Copy, Sig, Relu = AF.Copy, AF.Sigmoid, AF.Relu

    pers = ctx.enter_context(tc.tile_pool(name="pers", bufs=1))
    band = pers.tile([128, NELEM], F32, tag="band")
    up_bf = pers.tile([128, NPIX], BF16, tag="up_bf")
    feat_arm = pers.tile([128, NPIX], BF16, tag="feat_arm")

    consts = ctx.enter_context(tc.tile_pool(name="consts", bufs=1))

    def load_const(name):
        ap = d[name]
        t = consts.tile(list(ap.shape), ap.dtype, tag=name)
        nc.sync.dma_start(t[:], ap[:])
        return t

    att_lhsT = load_const("att_lhsT")
    arm_lhsT = load_const("arm_lhsT")
    om_lhsT_A = load_const("om_lhsT_A")
    om_lhsT_B = load_const("om_lhsT_B")
    om_bias = load_const("om_bias")
    wt_lhsT = load_const("wt_lhsT")
    sel_lhsT = load_const("sel_lhsT")
    dcn_bias = load_const("dcn_bias")
    lohi = load_const("lohi")

    band3 = band[:].rearrange("p (r c) -> p r c", r=BROW)

    # ================= S1-S3 (scoped pools) ==========================
    with ExitStack() as pre:
        s1 = pre.enter_context(tc.tile_pool(name="s1", bufs=1))
        s1psum = pre.enter_context(tc.tile_pool(name="s1psum", bufs=1, space="PSUM"))
        featl_half = s1.tile([128, NPIX], F32, tag="featl_half")
        nc.sync.dma_start(featl_half[:], d["feat_l"][:, 0:NPIX])
        featl_bf = s1.tile([128, NPIX], BF16, tag="featl_bf")

        partial = s1.tile([128, 8], F32, tag="partial")
        QH = NPIX // 4
        for i in range(4):
            nc.scalar.activation(featl_bf[:, i * QH:(i + 1) * QH],
                                 featl_half[:, i * QH:(i + 1) * QH],
                                 Copy, accum_out=partial[:, i:i + 1])
        oth = pre.enter_context(tc.tile_pool(name="oth", bufs=2))
        EH = NPIX // 4
        for i in range(4):
            t = oth.tile([128, EH], F32, tag="othl")
            nc.sync.dma_start(t[:], d["feat_l"][:, NPIX + i * EH:NPIX + (i + 1) * EH])
            scr = oth.tile([128, EH], BF16, tag="othscr")
            nc.scalar.activation(scr[:], t[:], Copy, accum_out=partial[:, 4 + i:5 + i])

        pooled = s1.tile([128, 1], F32, tag="pooled")
        nc.vector.tensor_reduce(pooled[:], partial[:], mybir.AxisListType.X, AX.add)
        att_psum = s1psum.tile([128, 2], F32, tag="attp")
        nc.tensor.matmul(att_psum[:, 0:1], att_lhsT[:], pooled[:],
                         start=True, stop=True)
        att = s1.tile([128, 1], F32, tag="att")
        nc.scalar.activation(att[:], att_psum[:, 0:1], Sig)
        attp1 = s1.tile([128, 1], F32, tag="attp1")
        nc.vector.tensor_scalar(attp1[:], att[:], 1.0, None, AX.add)
        armW = s1.tile([128, 128], BF16, tag="armW")
        nc.vector.tensor_scalar(armW[:], arm_lhsT[:], attp1[:, 0:1], None, AX.mult)

        # ---------------- S2: upsample -> band ------------------------
        s2 = pre.enter_context(tc.tile_pool(name="s2", bufs=1))
        feat_s = s2.tile([128, SROWS * WS], F32, tag="feat_s")
        nc.sync.dma_start(feat_s[:], d["feat_s_band"][:])
        nc.scalar.memzero(band[:])

        fs3 = feat_s[:].rearrange("p (r c) -> p r c", r=SROWS)
        XU = s2.tile([128, SROWS, 2 * WS], F32, tag="XU")
        xu3 = XU[:]
        t1 = s2.tile([128, SROWS, WS], F32, tag="ts")
        nc.vector.tensor_scalar(t1[:], fs3, 0.25, None, AX.mult)
        t13 = t1[:]

        nc.vector.scalar_tensor_tensor(
            xu3[:, :, 2::2], fs3[:, :, 1:], 0.75,
            t13[:, :, :WS - 1], AX.mult, AX.add)
        nc.vector.scalar_tensor_tensor(
            xu3[:, :, 1:2 * WS - 1:2], fs3[:, :, :WS - 1], 0.75,
            t13[:, :, 1:], AX.mult, AX.add)
        nc.vector.tensor_copy(xu3[:, :, 0:1], fs3[:, :, 0:1])
        nc.vector.tensor_copy(xu3[:, :, 2 * WS - 1:], fs3[:, :, WS - 1:])

        t2 = s2.tile([128, SROWS, 2 * WS], F32, tag="ts")
        nc.vector.tensor_scalar(t2[:], xu3, 0.25, None, AX.mult)
        t23 = t2[:]
        nc.vector.scalar_tensor_tensor(
            band3[:, 1::2, PAD:PAD + W], xu3[:, 2:39, :], 0.75,
            t23[:, 1:38, :], AX.mult, AX.add)
        nc.vector.scalar_tensor_tensor(
            band3[:, 0::2, PAD:PAD + W], xu3[:, 1:38, :], 0.75,
            t23[:, 2:39, :], AX.mult, AX.add)
        nc.scalar.activation(
            up_bf[:].rearrange("p (r c) -> p r c", r=64),
            band3[:, PAD:PAD + 64, PAD:PAD + W], Copy)

        # ---------------- S3: feat_arm GEMM ---------------------------
        s3psum = pre.enter_context(tc.tile_pool(name="s3psum", bufs=2, space="PSUM"))
        for i in range(NCHUNK):
            p = s3psum.tile([128, CHUNK], F32, tag="armp")
            nc.tensor.matmul(p[:], armW[:],
                             featl_bf[:, i * CHUNK:(i + 1) * CHUNK],
                             start=True, stop=True)
            nc.scalar.activation(feat_arm[:, i * CHUNK:(i + 1) * CHUNK], p[:], Copy)

    # ================= S4: main chunk loop ===========================
    ompsum = ctx.enter_context(tc.tile_pool(name="ompsum", bufs=1, space="PSUM"))
    w4psum = ctx.enter_context(tc.tile_pool(name="w4psum", bufs=1, space="PSUM"))
    valpsum = ctx.enter_context(tc.tile_pool(name="valpsum", bufs=1, space="PSUM"))
    cf = ctx.enter_context(tc.tile_pool(name="cf", bufs=1))
    gp = ctx.enter_context(tc.tile_pool(name="gp", bufs=2))
    outp = ctx.enter_context(tc.tile_pool(name="outp", bufs=2))

    band_g = band[:].rearrange("p (n d) -> p n d", d=1)

    for ci in range(NCHUNK):
        px = slice(ci * CHUNK, (ci + 1) * CHUNK)
        r0 = 4 * ci  # band row offset of this chunk (4 rows per 512 px)
        om = ompsum.tile([72, 3 * CHUNK], F32, tag="om")
        om3 = om[:].rearrange("p (q n) -> p q n", q=3)

        for q in range(3):
            nc.tensor.matmul(om3[:, q, :], om_lhsT_A[:, q * 72:(q + 1) * 72],
                             feat_arm[:, px], start=True, stop=False)
            nc.tensor.matmul(om3[:, q, :],
                             om_lhsT_B[:, q * 72:(q + 1) * 72],
                             up_bf[:, px], start=False, stop=True)

        # coef pipeline [72, CHUNK] f32
        oy = cf.tile([72, CHUNK], F32, tag="oy")
        nc.vector.tensor_scalar(oy[:], om3[:, 0, :], om_bias[:, 0:1], 1.0,
                                AX.add, AX.max)
        nc.vector.tensor_scalar(oy[:], oy[:], 7.0, None, AX.min)
        ry = cf.tile([72, CHUNK], F32, tag="ry")
        nc.vector.tensor_scalar(ry[:], oy[:], 8388608.0, None, AX.add)
        nc.vector.tensor_scalar(ry[:], ry[:], 8388608.0, None, AX.subtract)
        cy = cf.tile([72, CHUNK], F32, tag="cy")
        nc.vector.tensor_tensor(cy[:], ry[:], oy[:], AX.is_gt)
        ay = cf.tile([72, CHUNK], F32, tag="ay")
        nc.vector.tensor_tensor(ay[:], ry[:], cy[:], AX.subtract)
        fy = cf.tile([72, CHUNK], F32, tag="fy")
        nc.vector.tensor_tensor(fy[:], oy[:], ay[:], AX.subtract)

        ox = cf.tile([72, CHUNK], F32, tag="ox")
        nc.vector.tensor_scalar(ox[:], om3[:, 1, :], om_bias[:, 1:2], 1.0,
                                AX.add, AX.max)
        nc.vector.tensor_scalar(ox[:], ox[:], 7.0, None, AX.min)
        rx = cf.tile([72, CHUNK], F32, tag="rx")
        nc.vector.tensor_scalar(rx[:], ox[:], 8388608.0, None, AX.add)
        nc.vector.tensor_scalar(rx[:], rx[:], 8388608.0, None, AX.subtract)
        cx = cf.tile([72, CHUNK], F32, tag="cx")
        nc.vector.tensor_tensor(cx[:], rx[:], ox[:], AX.is_gt)
        axx = cf.tile([72, CHUNK], F32, tag="axx")
        nc.vector.tensor_tensor(axx[:], rx[:], cx[:], AX.subtract)
        fx = cf.tile([72, CHUNK], F32, tag="fx")
        nc.vector.tensor_tensor(fx[:], ox[:], axx[:], AX.subtract)

        msk = cf.tile([72, CHUNK], F32, tag="msk")
        nc.scalar.activation(msk[:], om3[:, 2, :], Sig, bias=om_bias[:, 2:3])

        w4 = cf.tile([72, 4 * CHUNK], BF16, tag="w4")
        w43 = w4[:].rearrange("p (j n) -> p j n", j=4)
        my = cf.tile([72, CHUNK], F32, tag="my")
        nc.vector.tensor_tensor(my[:], fy[:], msk[:], AX.mult)
        gy = cf.tile([72, CHUNK], F32, tag="gy")
        nc.vector.tensor_tensor(gy[:], msk[:], my[:], AX.subtract)
        nc.vector.tensor_tensor(w43[:, 1, :], gy[:], fx[:], AX.mult)
        nc.vector.tensor_tensor(w43[:, 0, :], gy[:], w43[:, 1, :], AX.subtract)
        nc.vector.tensor_tensor(w43[:, 3, :], my[:], fx[:], AX.mult)
        nc.vector.tensor_tensor(w43[:, 2, :], my[:], w43[:, 3, :], AX.subtract)

        # indices: idx4T layout [72, (j, w=16, k=32)]
        basep_t = cf.tile([72, CHUNK], F32, tag="basep_t")
        nc.sync.dma_start(basep_t[:], d["basep"][:, px])
        idxC = cf.tile([72, CHUNK], F32, tag="idxC")
        nc.vector.scalar_tensor_tensor(idxC[:], ay[:], float(BCOL), axx[:],
                                       AX.mult, AX.add)
        # idx4T storage [72, (w=16, j=4, k=32)]
        idx4T = cf.tile([72, 4 * CHUNK], F32, tag="idx4T")
        i43 = idx4T[:].rearrange("p (w j k) -> p w j k", w=16, j=4)
        nc.vector.tensor_tensor(
            i43[:, :, 0, :].rearrange("p w k -> p k w"),
            idxC[:].rearrange("p (k w) -> p k w", k=32),
            basep_t[:].rearrange("p (k w) -> p k w", k=32), AX.add)
        for j, dj in ((1, 1.0), (2, float(BCOL)), (3, float(BCOL + 1))):
            nc.vector.tensor_scalar(i43[:, :, j, :], i43[:, :, 0, :], dj, None, AX.add)
        nc.vector.tensor_scalar(idx4T[:], idx4T[:], lohi[:, 0:1], lohi[:, 1:2],
                                AX.max, AX.min)
        idx16 = cf.tile([72, 4 * CHUNK], I16, tag="idx16")
        nc.vector.tensor_copy(idx16[:], idx4T[:])

        # wrap DMAs: wrapped[16g+w, t*128 + j*32 + k] = idx16[8t+g, j, w, k]
        wrapped = cf.tile([128, T * 128], I16, tag="wrapped")
        for t in range(T):
            nc.sync.dma_start(
                wrapped[:, t * 128:(t + 1) * 128],
                idx16[8 * t:8 * t + 8, :].rearrange(
                    "p (w jk) -> p w jk", w=16))

        val_ps = valpsum.tile([128, CHUNK], F32, tag="valps")
        for t in range(T):
            gath = gp.tile([128, 4 * CHUNK], F32, tag="gath")
            nc.gpsimd.ap_gather(
                gath[:].rearrange("p (n d) -> p n d", d=1), band_g,
                wrapped[:, t * 128:(t + 1) * 128],
                channels=128, num_elems=NELEM, d=1, num_idxs=4 * CHUNK)
            w4r_ps = w4psum.tile([128, 4 * CHUNK], F32, tag="w4rps")
            w4v = w4[:].rearrange("p (j n) -> p j n", j=4)
            w4rv = w4r_ps[:].rearrange("p (j n) -> p j n", j=4)
            for j in range(4):
                nc.tensor.matmul(w4rv[:, j, :],
                                 sel_lhsT[:, t * 128:(t + 1) * 128],
                                 w4v[:, j, :], start=True, stop=True)
            w4r = gp.tile([128, 4 * CHUNK], F32, tag="w4r")
            nc.scalar.activation(w4r[:], w4r_ps[:], Copy)
            m = gp.tile([128, 4 * CHUNK], BF16, tag="m")
            nc.vector.tensor_tensor(m[:], gath[:], w4r[:], AX.mult)
            a1 = gp.tile([128, 2 * CHUNK], BF16, tag="a1")
            nc.vector.tensor_tensor(a1[:], m[:, :2 * CHUNK], m[:, 2 * CHUNK:], AX.add)
            vt = gp.tile([128, CHUNK], BF16, tag="vt")
            nc.vector.tensor_tensor(vt[:], a1[:, :CHUNK], a1[:, CHUNK:], AX.add)
            nc.tensor.matmul(val_ps[:], wt_lhsT[:, t * 128:(t + 1) * 128], vt[:],
                             start=(t == 0), stop=(t == T - 1))

        tmp = outp.tile([128, CHUNK], F32, tag="tmp")
        nc.scalar.activation(tmp[:], val_ps[:], Relu, bias=dcn_bias[:, 0:1])
        oc = outp.tile([128, CHUNK], F32, tag="oc")
        nc.vector.tensor_tensor(oc[:], tmp[:], feat_arm[:, px], AX.add)
        nc.sync.dma_start(y[:, px], oc[:])


# --------------------------------------------------------------------------
# public entry point
# --------------------------------------------------------------------------
def kernel(feat_l, feat_s, fsm_atten_w, fsm_conv_w, om_w, om_b, dcn_w, dcn_b):
    global _PROG
    _lazy_imports()
    feat_l = np.asarray(feat_l, np.float32)
    feat_s = np.asarray(feat_s, np.float32)
    consts = _make_consts(np.asarray(fsm_atten_w, np.float32),
                          np.asarray(fsm_conv_w, np.float32),
                          np.asarray(om_w, np.float32),
                          np.asarray(om_b, np.float32),
                          np.asarray(dcn_w, np.float32),
                          np.asarray(dcn_b, np.float32))
    if _PROG is None:
        _PROG = _build()
    inputs = dict(feat_l=feat_l, feat_s=feat_s)
    in_maps = [_per_core_inputs(inputs, consts, c) for c in range(N_CORES)]
    res = _BASS["run"](_PROG, in_maps, core_ids=list(range(N_CORES)))
    out = np.zeros((B, NC, H, W), np.float32)
    for c in range(N_CORES):
        b, h0 = c // 2, 64 * (c % 2)
        out[b, :, h0:h0 + 64, :] = res.results[c]["y"].reshape(NC, 64, W)
    return out
